# revision 20
# baseline (speedup 1.0000x reference)
"""Block-sparse attention (CABAttention) Trainium2 kernel — v3.

Sharding: 8 cores = 2 batches x 4 head-groups (4 heads each).
Per core: qkv projection (fp16 in, fp32 PSUM), top-2+diag block-sparse
attention (fp16 value path), output projection (row-parallel partial
sums, f16, host-summed + bias).

v3: PE-dynamic architecture (dynamic block offsets consumed as
register-offset moving operands on the tensor engine — the only
stable high-rate dynamic path: SP-issued dynamic DMAs cost ~730ns
sequencer time each, and DVE/ACT reg_loads hang the device). vs the
v1 baseline: ONE 4-register load per (pair, qblock) iteration instead
of four serialized single loads, fp16 qkv projection, anchor distance
6 (not 3), and the output projection interleaved into the attention
loop so the PE stays dense and the output DMA overlaps compute.
Block selection (top-2 of coarse block-mean scores) runs on host in
float64 and is passed as index inputs.
"""
import sys

sys.path.insert(0, "/opt/trn_rl_repo")

import numpy as np

import concourse.bass as bass
import concourse.mybir as mybir
import concourse.tile as tile
from concourse import bacc
from concourse.bass import ds
from concourse.bass_utils import run_bass_kernel_spmd
from concourse.masks import make_identity

F32 = mybir.dt.float32
F16 = mybir.dt.float16
I32 = mybir.dt.int32
ET = mybir.EngineType

DIM = 1024
H = 16
HD = 64
BS = 64
N = 2048
B = 2
M = N // BS            # 32 blocks
SCALE = HD ** -0.5
NCORES = 8
HPC = H // (NCORES // B)   # 4 heads per core

_NC_CACHE = None
LAST_RESULTS = None


def build_kernel():
    import os
    anchor_d = int(os.environ.get("ANCHOR_D", "8"))
    nc = bacc.Bacc(None)
    xt_d = nc.dram_tensor("xt", [DIM, N], F16, kind="ExternalInput")
    wq_d = nc.dram_tensor("wq", [DIM, 768], F16, kind="ExternalInput")
    pw_d = nc.dram_tensor("pw", [256, DIM], F16, kind="ExternalInput")
    idx_d = nc.dram_tensor("selidx", [1, 256], I32, kind="ExternalInput")
    wb_d = nc.dram_tensor("wbias", [128, 64], F32, kind="ExternalInput")
    y_d = nc.dram_tensor("y", [N, DIM], F16, kind="ExternalOutput")

    with tile.TileContext(nc) as tc:
        with tc.tile_pool(name="big", bufs=1) as big, \
             tc.tile_pool(name="wrk", bufs=8) as wrk:

            # ---- persistent SBUF tensors ----
            xt = big.tile([128, 8, N], F16)           # x^T, feature-major
            wq = big.tile([128, 8, 768], F16)         # qkv weights^T
            pwt = big.tile([128, 2, DIM], F16)        # proj weights
            idx = big.tile([1, 256], I32)
            wb = big.tile([128, 64], F32)
            qT = [big.tile([128, N], F16, name=f"qT{i}") for i in range(2)]
            kkT = [big.tile([128, N], F16, name=f"kkT{i}") for i in range(2)]
            vvT = [big.tile([128, N], F16, name=f"vvT{i}") for i in range(2)]
            vdAB = [big.tile([64, 2, N], F16, name=f"vdAB{i}")
                    for i in range(2)]
            vdA = [vdAB[i][:, 0, :] for i in range(2)]
            vdB = [vdAB[i][:, 1, :] for i in range(2)]
            outT = [big.tile([128, N], F16, name=f"outT{i}") for i in range(2)]
            qTB = [big.tile([64, N], F16, name=f"qTB{i}") for i in range(2)]
            kkTB = [big.tile([64, N], F16, name=f"kkTB{i}") for i in range(2)]
            identf = big.tile([128, 128], F32)
            ident = big.tile([128, 128], F16)

            # ---- input DMAs (split for pipelining) ----
            xt_v = xt_d[:].rearrange("(a p) n -> p a n", p=128)
            wq_v = wq_d[:].rearrange("(a p) n -> p a n", p=128)
            pw_v = pw_d[:].rearrange("(a p) n -> p a n", p=128)
            for k in range(8):
                nc.sync.dma_start(xt[:, k, :], xt_v[:, k, :])
                nc.sync.dma_start(wq[:, k, :], wq_v[:, k, :])
            nc.sync.dma_start(pwt[:], pw_v[:])
            idx_dma = nc.sync.dma_start(idx[:], idx_d[:])
            nc.sync.dma_start(wb[:], wb_d[:])

            make_identity(nc, identf[:])
            nc.vector.tensor_copy(ident[:], identf[:])

            # ---- qkv projection: fp16 inputs, fp32 PSUM over 8 K-chunks ----
            # M-tiles: 0,1 -> qT pair0/1; 2,4 -> kkT; 3,5 -> vvT
            tgt = [qT[0], qT[1], kkT[0], vvT[0], kkT[1], vvT[1]]
            with tc.tile_pool(name="qkps", bufs=4, space="PSUM") as qkps:
                for mt in range(6):
                    for nt in range(4):
                        ps = qkps.tile([128, 512], F32)
                        for k in range(8):
                            nc.tensor.matmul(
                                ps[:], lhsT=wq[:, k, mt * 128:(mt + 1) * 128],
                                rhs=xt[:, k, nt * 512:(nt + 1) * 512],
                                start=(k == 0), stop=(k == 7))
                        nc.vector.tensor_copy(
                            tgt[mt][:, nt * 512:(nt + 1) * 512], ps[:])

            # dynamic-offset matmul operands must sit at partition base 0:
            # make base-0 copies of the head-B halves
            for p in range(2):
                nc.sync.dma_start(qTB[p][:], qT[p][64:128, :])
                nc.sync.dma_start(kkTB[p][:], kkT[p][64:128, :])

            # ---- v_dup: per pair, transpose vvT blocks to keys-major ----
            with tc.tile_pool(name="vtps", bufs=2, space="PSUM") as vtps:
                for p in range(2):
                    for j in range(M):
                        tp = vtps.tile([64, 128], F16)
                        nc.tensor.transpose(
                            tp[:], vvT[p][:, j * 64:(j + 1) * 64], ident[:])
                        nc.vector.tensor_copy(
                            vdAB[p][:, :, j * 64:(j + 1) * 64],
                            tp[:].rearrange("p (h c) -> p h c", h=2))

            # ---- block-sparse attention + interleaved projection ----
            # software-pipelined: stage A (scores+softmax), B (transpose
            # probs), C (AV), D (out transpose) of four consecutive
            # iterations interleave so the PE never waits on the
            # DVE/ACT softmax round-trips.
            with tc.tile_pool(name="fsps", bufs=3, space="PSUM") as fsps, \
                 tc.tile_pool(name="hsps", bufs=3, space="PSUM") as hsps, \
                 tc.tile_pool(name="ypsp", bufs=2, space="PSUM") as ypsp:
                anchors = {}
                st = {}
                NT = 2 * M

                def stage_a(t):
                    qb, p = t // 2, t % 2
                    qs = slice(qb * 64, (qb + 1) * 64)
                    base = p * 128 + qb * 4
                    loads, offs = nc.values_load_multi_w_load_instructions(
                        idx[0:1, base:base + 4], engines=[ET.PE],
                        min_val=0, max_val=N - 64,
                        skip_runtime_bounds_check=True)
                    for li in loads:
                        tile.add_dep_helper(li.ins, idx_dma.ins, sync=True,
                                            reason="idx dma -> pe regs")
                        if t >= anchor_d:
                            tile.add_dep_helper(
                                li.ins, anchors[t - anchor_d].ins, sync=False,
                                reason="bound PE register live range")
                    oA1, oA2, oB1, oB2 = offs
                    fs = fsps.tile([128, 320], F32, tag="fs")
                    sps = fs[:, 0:192]
                    for s, (oa, ob) in enumerate(
                            [(oA1, oB1), (oA2, oB2),
                             (qb * 64, qb * 64)]):
                        cs = slice(s * 64, (s + 1) * 64)
                        nc.tensor.matmul(
                            sps[0:64, cs], lhsT=qT[p][0:64, qs],
                            rhs=kkT[p][0:64, ds(oa, 64)],
                            start=True, stop=True)
                        nc.tensor.matmul(
                            sps[64:128, cs], lhsT=qTB[p][:, qs],
                            rhs=kkTB[p][:, ds(ob, 64)],
                            start=True, stop=True,
                            skip_group_check=True,
                            tile_position=(0, 64))
                    # mask duplicated diag slot (bias -30000 -> exp 0)
                    nc.scalar.activation(
                        sps[:, 128:192], sps[:, 128:192],
                        mybir.ActivationFunctionType.Identity,
                        bias=wb[:, p * 32 + qb:p * 32 + qb + 1])
                    pu = wrk.tile([128, 192], F16, tag="pu")
                    den = wrk.tile([128, 1], F32, tag="den")
                    nc.scalar.activation(pu[:], sps[:],
                                         mybir.ActivationFunctionType.Exp,
                                         accum_out=den[:])
                    rden = wrk.tile([128, 1], F32, tag="rden")
                    nc.vector.reciprocal(rden[:], den[:])
                    pr = wrk.tile([128, 192], F16, tag="pr")
                    nc.vector.tensor_scalar(pr[:], pu[:], rden[:, 0:1], None,
                                            op0=mybir.AluOpType.mult)
                    st[t] = {"offs": offs, "fs": fs, "pr": pr}

                def stage_b(t):
                    s_ = st[t]
                    hs = hsps.tile([128, 448], F16, tag="hs")
                    pt = hs[0:64, 0:384]
                    for s in range(3):
                        nc.tensor.transpose(
                            pt[:, s * 128:(s + 1) * 128],
                            s_["pr"][:, s * 64:(s + 1) * 64], ident[:])
                    pts = wrk.tile([64, 384], F16, tag="pts")
                    nc.vector.tensor_copy(pts[:], pt[:])
                    s_["hs"] = hs
                    s_["pts"] = pts

                def stage_c(t):
                    qb, p = t // 2, t % 2
                    s_ = st[t]
                    oA1, oA2, oB1, oB2 = s_["offs"]
                    pts = s_["pts"]
                    avpAB = s_["fs"][0:64, 192:320]
                    for s, o in enumerate([oA1, oA2, qb * 64]):
                        nc.tensor.matmul(
                            avpAB[:, 0:64],
                            lhsT=pts[:, s * 128:s * 128 + 64],
                            rhs=vdA[p][:, ds(o, 64)],
                            start=(s == 0), stop=(s == 2))
                    for s, o in enumerate([oB1, oB2, qb * 64]):
                        mi = nc.tensor.matmul(
                            avpAB[:, 64:128],
                            lhsT=pts[:, s * 128 + 64:s * 128 + 128],
                            rhs=vdB[p][:, ds(o, 64)],
                            start=(s == 0), stop=(s == 2))
                    anchors[t] = mi
                    av_sb = wrk.tile([64, 128], F16, tag="av_sb")
                    nc.scalar.copy(av_sb[:], avpAB[:])
                    s_["av_sb"] = av_sb

                def stage_d(t):
                    qb, p = t // 2, t % 2
                    qs = slice(qb * 64, (qb + 1) * 64)
                    s_ = st.pop(t)
                    otp = s_["hs"][:, 384:448]
                    nc.tensor.transpose(otp[:], s_["av_sb"][:],
                                        ident[0:64, 0:64])
                    nc.vector.tensor_copy(outT[p][:, qs], otp[:])

                def proj_half(tt, nt):
                    ts_ = slice(tt * 128, (tt + 1) * 128)
                    ns = slice(nt * 512, (nt + 1) * 512)
                    yp = ypsp.tile([128, 512], F32)
                    nc.tensor.matmul(yp[:], lhsT=outT[0][:, ts_],
                                     rhs=pwt[:, 0, ns],
                                     start=True, stop=False)
                    nc.tensor.matmul(yp[:], lhsT=outT[1][:, ts_],
                                     rhs=pwt[:, 1, ns],
                                     start=False, stop=True)
                    ys = wrk.tile([128, 512], F16, tag="ys")
                    nc.vector.tensor_copy(ys[:], yp[:])
                    nc.sync.dma_start(y_d[ts_, ns], ys[:])

                # iteration order: p inner-fast would split pairs; use
                # t = qb*2 + p so both pairs of a token tile finish on
                # consecutive t (proj after t = 4*tt+3 completes stage D)
                for w in range(NT + 3):
                    if w < NT:
                        stage_a(w)
                    if 0 <= w - 1 < NT:
                        stage_b(w - 1)
                    if 0 <= w - 2 < NT:
                        stage_c(w - 2)
                    if 0 <= w - 3 < NT:
                        td = w - 3
                        stage_d(td)
                        if td % 4 == 3:
                            proj_half(td // 4, 0)
                        elif td % 4 == 1 and td >= 5:
                            proj_half((td - 5) // 4, 1)
                proj_half(M // 2 - 1, 1)

    nc.finalize()
    return nc


def _host_prep(x, qkv_w, proj_w):
    """Per-core input maps + block selection (float64, matches fp32 ref)."""
    in_maps = []
    x64 = x.astype(np.float64)
    for core in range(NCORES):
        b = core // (NCORES // B)
        hg = core % (NCORES // B)
        heads = [hg * HPC + i for i in range(HPC)]

        xt = np.ascontiguousarray(x[b].T).astype(np.float16)

        wqkvT = np.empty((DIM, 768), np.float32)
        for p in range(2):
            hA, hB = heads[2 * p], heads[2 * p + 1]
            wqkvT[:, p*128:p*128+64] = qkv_w[hA*64:(hA+1)*64].T * SCALE
            wqkvT[:, p*128+64:p*128+128] = qkv_w[hB*64:(hB+1)*64].T * SCALE
            kbase = 256 + p * 256
            wqkvT[:, kbase:kbase+64] = qkv_w[DIM+hA*64:DIM+(hA+1)*64].T
            wqkvT[:, kbase+64:kbase+128] = qkv_w[DIM+hB*64:DIM+(hB+1)*64].T
            vbase = kbase + 128
            wqkvT[:, vbase:vbase+64] = qkv_w[2*DIM+hA*64:2*DIM+(hA+1)*64].T
            wqkvT[:, vbase+64:vbase+128] = qkv_w[2*DIM+hB*64:2*DIM+(hB+1)*64].T

        pw = np.ascontiguousarray(
            proj_w[:, heads[0]*64:(heads[-1]+1)*64].T).astype(np.float16)

        # float64 selection (matches fp32 reference ordering w/ margin)
        xb = x64[b].reshape(M, BS, DIM).mean(axis=1)
        selidx = np.zeros((1, 256), np.int32)
        wbias = np.zeros((128, 64), np.float32)
        for p in range(2):
            for hip in range(2):
                h = heads[2 * p + hip]
                qb_ = xb @ qkv_w[h*64:(h+1)*64].T.astype(np.float64)
                kb_ = xb @ qkv_w[DIM+h*64:DIM+(h+1)*64].T.astype(np.float64)
                c = qb_ @ kb_.T
                for i in range(M):
                    order = np.argsort(-c[i], kind="stable")
                    i1, i2 = int(order[0]), int(order[1])
                    col = p * 128 + i * 4 + hip * 2
                    selidx[0, col] = i1 * 64
                    selidx[0, col + 1] = i2 * 64
                    if i == i1 or i == i2:
                        wbias[hip*64:(hip+1)*64, p*32+i] = -30000.0
        in_maps.append({"xt": xt, "wq": wqkvT.astype(np.float16), "pw": pw,
                        "selidx": selidx, "wbias": wbias})
    return in_maps


def kernel(x, qkv_w, proj_w, proj_b):
    global _NC_CACHE, LAST_RESULTS
    x = np.asarray(x, np.float32)
    qkv_w = np.asarray(qkv_w, np.float32)
    proj_w = np.asarray(proj_w, np.float32)
    proj_b = np.asarray(proj_b, np.float32)

    if _NC_CACHE is None:
        _NC_CACHE = build_kernel()
    nc = _NC_CACHE

    in_maps = _host_prep(x, qkv_w, proj_w)
    res = run_bass_kernel_spmd(nc, in_maps, list(range(NCORES)))
    LAST_RESULTS = res

    out = np.zeros((B, N, DIM), np.float32)
    for core in range(NCORES):
        out[core // (NCORES // B)] += res.results[core]["y"].astype(np.float32)
    out += proj_b[None, None, :]
    return out


# revision 21
# speedup vs baseline: 1.0343x; 1.0343x over previous
"""Block-sparse attention (CABAttention) Trainium2 kernel — v3.

Sharding: 8 cores = 2 batches x 4 head-groups (4 heads each).
Per core: qkv projection (fp16 in, fp32 PSUM), top-2+diag block-sparse
attention (fp16 value path), output projection (row-parallel partial
sums, f16, host-summed + bias).

v3: PE-dynamic architecture (dynamic block offsets consumed as
register-offset moving operands on the tensor engine — the only
stable high-rate dynamic path: SP-issued dynamic DMAs cost ~730ns
sequencer time each, and DVE/ACT reg_loads hang the device). vs the
v1 baseline: ONE 4-register load per (pair, qblock) iteration instead
of four serialized single loads, fp16 qkv projection, anchor distance
6 (not 3), and the output projection interleaved into the attention
loop so the PE stays dense and the output DMA overlaps compute.
Block selection (top-2 of coarse block-mean scores) runs on host in
float64 and is passed as index inputs.
"""
import sys

sys.path.insert(0, "/opt/trn_rl_repo")

import numpy as np

import concourse.bass as bass
import concourse.mybir as mybir
import concourse.tile as tile
from concourse import bacc
from concourse.bass import ds
from concourse.bass_utils import run_bass_kernel_spmd
from concourse.masks import make_identity

F32 = mybir.dt.float32
F16 = mybir.dt.float16
I32 = mybir.dt.int32
ET = mybir.EngineType

DIM = 1024
H = 16
HD = 64
BS = 64
N = 2048
B = 2
M = N // BS            # 32 blocks
SCALE = HD ** -0.5
NCORES = 8
HPC = H // (NCORES // B)   # 4 heads per core

_NC_CACHE = None
LAST_RESULTS = None


def build_kernel():
    import os
    anchor_d = int(os.environ.get("ANCHOR_D", "8"))
    nc = bacc.Bacc(None)
    xt_d = nc.dram_tensor("xt", [DIM, N], F16, kind="ExternalInput")
    wq_d = nc.dram_tensor("wq", [DIM, 768], F16, kind="ExternalInput")
    pw_d = nc.dram_tensor("pw", [256, DIM], F16, kind="ExternalInput")
    idx_d = nc.dram_tensor("selidx", [1, 256], I32, kind="ExternalInput")
    wb_d = nc.dram_tensor("wbias", [128, 64], F32, kind="ExternalInput")
    y_d = nc.dram_tensor("y", [N, DIM], F16, kind="ExternalOutput")

    with tile.TileContext(nc) as tc:
        with tc.tile_pool(name="big", bufs=1) as big, \
             tc.tile_pool(name="wrk", bufs=8) as wrk:

            # ---- persistent SBUF tensors ----
            xt = big.tile([128, 8, N], F16)           # x^T, feature-major
            wq = big.tile([128, 8, 768], F16)         # qkv weights^T
            pwt = big.tile([128, 2, DIM], F16)        # proj weights
            idx = big.tile([1, 256], I32)
            wb = big.tile([128, 64], F32)
            qT = [big.tile([128, N], F16, name=f"qT{i}") for i in range(2)]
            kkT = [big.tile([128, N], F16, name=f"kkT{i}") for i in range(2)]
            vvT = [big.tile([128, N], F16, name=f"vvT{i}") for i in range(2)]
            vdAB = [big.tile([64, 2, N], F16, name=f"vdAB{i}")
                    for i in range(2)]
            vdA = [vdAB[i][:, 0, :] for i in range(2)]
            vdB = [vdAB[i][:, 1, :] for i in range(2)]
            outT = [big.tile([128, N], F16, name=f"outT{i}") for i in range(2)]
            qTB = [big.tile([64, N], F16, name=f"qTB{i}") for i in range(2)]
            kkTB = [big.tile([64, N], F16, name=f"kkTB{i}") for i in range(2)]
            identf = big.tile([128, 128], F32)
            ident = big.tile([128, 128], F16)

            # ---- input DMAs (split for pipelining) ----
            xt_v = xt_d[:].rearrange("(a p) n -> p a n", p=128)
            wq_v = wq_d[:].rearrange("(a p) n -> p a n", p=128)
            pw_v = pw_d[:].rearrange("(a p) n -> p a n", p=128)
            for k in range(8):
                nc.sync.dma_start(xt[:, k, :], xt_v[:, k, :])
                nc.sync.dma_start(wq[:, k, :], wq_v[:, k, :])
            nc.sync.dma_start(pwt[:], pw_v[:])
            idx_dma = nc.sync.dma_start(idx[:], idx_d[:])
            nc.sync.dma_start(wb[:], wb_d[:])

            make_identity(nc, identf[:])
            nc.vector.tensor_copy(ident[:], identf[:])

            # ---- qkv projection: fp16 inputs, fp32 PSUM over 8 K-chunks ----
            # M-tiles: 0,1 -> qT pair0/1; 2,4 -> kkT; 3,5 -> vvT
            tgt = [qT[0], qT[1], kkT[0], vvT[0], kkT[1], vvT[1]]
            with tc.tile_pool(name="qkps", bufs=4, space="PSUM") as qkps:
                for mt in range(6):
                    for nt in range(4):
                        ps = qkps.tile([128, 512], F32)
                        for k in range(8):
                            nc.tensor.matmul(
                                ps[:], lhsT=wq[:, k, mt * 128:(mt + 1) * 128],
                                rhs=xt[:, k, nt * 512:(nt + 1) * 512],
                                start=(k == 0), stop=(k == 7))
                        nc.vector.tensor_copy(
                            tgt[mt][:, nt * 512:(nt + 1) * 512], ps[:])

            # dynamic-offset matmul operands must sit at partition base 0:
            # make base-0 copies of the head-B halves
            for p in range(2):
                nc.sync.dma_start(qTB[p][:], qT[p][64:128, :])
                nc.sync.dma_start(kkTB[p][:], kkT[p][64:128, :])

            # ---- v_dup: per pair, transpose vvT blocks to keys-major ----
            with tc.tile_pool(name="vtps", bufs=2, space="PSUM") as vtps:
                for p in range(2):
                    for j in range(M):
                        tp = vtps.tile([64, 128], F16)
                        nc.tensor.transpose(
                            tp[:], vvT[p][:, j * 64:(j + 1) * 64], ident[:])
                        nc.vector.tensor_copy(
                            vdAB[p][:, :, j * 64:(j + 1) * 64],
                            tp[:].rearrange("p (h c) -> p h c", h=2))

            # ---- block-sparse attention + interleaved projection ----
            # software-pipelined: stage A (scores+softmax), B (transpose
            # probs), C (AV), D (out transpose) of four consecutive
            # iterations interleave so the PE never waits on the
            # DVE/ACT softmax round-trips.
            with tc.tile_pool(name="fsps", bufs=4, space="PSUM") as fsps, \
                 tc.tile_pool(name="hsps", bufs=3, space="PSUM") as hsps, \
                 tc.tile_pool(name="ypsp", bufs=1, space="PSUM") as ypsp:
                anchors = {}
                st = {}
                NT = 2 * M

                def stage_a(t):
                    qb, p = t // 2, t % 2
                    qs = slice(qb * 64, (qb + 1) * 64)
                    base = p * 128 + qb * 4
                    loads, offs = nc.values_load_multi_w_load_instructions(
                        idx[0:1, base:base + 4], engines=[ET.PE],
                        min_val=0, max_val=N - 64,
                        skip_runtime_bounds_check=True)
                    for li in loads:
                        tile.add_dep_helper(li.ins, idx_dma.ins, sync=True,
                                            reason="idx dma -> pe regs")
                        if t >= anchor_d:
                            tile.add_dep_helper(
                                li.ins, anchors[t - anchor_d].ins, sync=False,
                                reason="bound PE register live range")
                    oA1, oA2, oB1, oB2 = offs
                    fs = fsps.tile([128, 320], F32, tag="fs")
                    sps = fs[:, 0:192]
                    for s, (oa, ob) in enumerate(
                            [(oA1, oB1), (oA2, oB2),
                             (qb * 64, qb * 64)]):
                        cs = slice(s * 64, (s + 1) * 64)
                        nc.tensor.matmul(
                            sps[0:64, cs], lhsT=qT[p][0:64, qs],
                            rhs=kkT[p][0:64, ds(oa, 64)],
                            start=True, stop=True)
                        nc.tensor.matmul(
                            sps[64:128, cs], lhsT=qTB[p][:, qs],
                            rhs=kkTB[p][:, ds(ob, 64)],
                            start=True, stop=True,
                            skip_group_check=True,
                            tile_position=(0, 64))
                    # mask duplicated diag slot (bias -30000 -> exp 0)
                    nc.scalar.activation(
                        sps[:, 128:192], sps[:, 128:192],
                        mybir.ActivationFunctionType.Identity,
                        bias=wb[:, p * 32 + qb:p * 32 + qb + 1])
                    pu = wrk.tile([128, 192], F16, tag="pu")
                    den = wrk.tile([128, 1], F32, tag="den")
                    nc.scalar.activation(pu[:], sps[:],
                                         mybir.ActivationFunctionType.Exp,
                                         accum_out=den[:])
                    rden = wrk.tile([128, 1], F32, tag="rden")
                    nc.vector.reciprocal(rden[:], den[:])
                    pr = wrk.tile([128, 192], F16, tag="pr")
                    nc.vector.tensor_scalar(pr[:], pu[:], rden[:, 0:1], None,
                                            op0=mybir.AluOpType.mult)
                    st[t] = {"offs": offs, "fs": fs, "pr": pr}

                def stage_b(t):
                    s_ = st[t]
                    hs = hsps.tile([128, 448], F16, tag="hs")
                    pt = hs[0:64, 0:384]
                    for s in range(3):
                        nc.tensor.transpose(
                            pt[:, s * 128:(s + 1) * 128],
                            s_["pr"][:, s * 64:(s + 1) * 64], ident[:])
                    pts = wrk.tile([64, 384], F16, tag="pts")
                    nc.vector.tensor_copy(pts[:], pt[:])
                    s_["hs"] = hs
                    s_["pts"] = pts

                def stage_c(t):
                    qb, p = t // 2, t % 2
                    s_ = st[t]
                    oA1, oA2, oB1, oB2 = s_["offs"]
                    pts = s_["pts"]
                    avpAB = s_["fs"][0:64, 192:320]
                    for s, o in enumerate([oA1, oA2, qb * 64]):
                        nc.tensor.matmul(
                            avpAB[:, 0:64],
                            lhsT=pts[:, s * 128:s * 128 + 64],
                            rhs=vdA[p][:, ds(o, 64)],
                            start=(s == 0), stop=(s == 2))
                    for s, o in enumerate([oB1, oB2, qb * 64]):
                        mi = nc.tensor.matmul(
                            avpAB[:, 64:128],
                            lhsT=pts[:, s * 128 + 64:s * 128 + 128],
                            rhs=vdB[p][:, ds(o, 64)],
                            start=(s == 0), stop=(s == 2))
                    anchors[t] = mi
                    av_sb = wrk.tile([64, 128], F16, tag="av_sb")
                    nc.scalar.copy(av_sb[:], avpAB[:])
                    s_["av_sb"] = av_sb

                def stage_d(t):
                    qb, p = t // 2, t % 2
                    qs = slice(qb * 64, (qb + 1) * 64)
                    s_ = st.pop(t)
                    otp = s_["hs"][:, 384:448]
                    nc.tensor.transpose(otp[:], s_["av_sb"][:],
                                        ident[0:64, 0:64])
                    nc.vector.tensor_copy(outT[p][:, qs], otp[:])

                def proj_half(tt, nt):
                    ts_ = slice(tt * 128, (tt + 1) * 128)
                    ns = slice(nt * 512, (nt + 1) * 512)
                    yp = ypsp.tile([128, 512], F32)
                    nc.tensor.matmul(yp[:], lhsT=outT[0][:, ts_],
                                     rhs=pwt[:, 0, ns],
                                     start=True, stop=False)
                    nc.tensor.matmul(yp[:], lhsT=outT[1][:, ts_],
                                     rhs=pwt[:, 1, ns],
                                     start=False, stop=True)
                    ys = wrk.tile([128, 512], F16, tag="ys")
                    nc.vector.tensor_copy(ys[:], yp[:])
                    nc.sync.dma_start(y_d[ts_, ns], ys[:])

                # iteration order: p inner-fast would split pairs; use
                # t = qb*2 + p so both pairs of a token tile finish on
                # consecutive t (proj after t = 4*tt+3 completes stage D)
                for w in range(NT + 3):
                    if w < NT:
                        stage_a(w)
                    if 0 <= w - 1 < NT:
                        stage_b(w - 1)
                    if 0 <= w - 2 < NT:
                        stage_c(w - 2)
                    if 0 <= w - 3 < NT:
                        td = w - 3
                        stage_d(td)
                        if td % 4 == 3:
                            proj_half(td // 4, 0)
                        elif td % 4 == 1 and td >= 5:
                            proj_half((td - 5) // 4, 1)
                proj_half(M // 2 - 1, 1)

    nc.finalize()
    return nc


def _host_prep(x, qkv_w, proj_w):
    """Per-core input maps + block selection (float64, matches fp32 ref)."""
    in_maps = []
    x64 = x.astype(np.float64)
    for core in range(NCORES):
        b = core // (NCORES // B)
        hg = core % (NCORES // B)
        heads = [hg * HPC + i for i in range(HPC)]

        xt = np.ascontiguousarray(x[b].T).astype(np.float16)

        wqkvT = np.empty((DIM, 768), np.float32)
        for p in range(2):
            hA, hB = heads[2 * p], heads[2 * p + 1]
            wqkvT[:, p*128:p*128+64] = qkv_w[hA*64:(hA+1)*64].T * SCALE
            wqkvT[:, p*128+64:p*128+128] = qkv_w[hB*64:(hB+1)*64].T * SCALE
            kbase = 256 + p * 256
            wqkvT[:, kbase:kbase+64] = qkv_w[DIM+hA*64:DIM+(hA+1)*64].T
            wqkvT[:, kbase+64:kbase+128] = qkv_w[DIM+hB*64:DIM+(hB+1)*64].T
            vbase = kbase + 128
            wqkvT[:, vbase:vbase+64] = qkv_w[2*DIM+hA*64:2*DIM+(hA+1)*64].T
            wqkvT[:, vbase+64:vbase+128] = qkv_w[2*DIM+hB*64:2*DIM+(hB+1)*64].T

        pw = np.ascontiguousarray(
            proj_w[:, heads[0]*64:(heads[-1]+1)*64].T).astype(np.float16)

        # float64 selection (matches fp32 reference ordering w/ margin)
        xb = x64[b].reshape(M, BS, DIM).mean(axis=1)
        selidx = np.zeros((1, 256), np.int32)
        wbias = np.zeros((128, 64), np.float32)
        for p in range(2):
            for hip in range(2):
                h = heads[2 * p + hip]
                qb_ = xb @ qkv_w[h*64:(h+1)*64].T.astype(np.float64)
                kb_ = xb @ qkv_w[DIM+h*64:DIM+(h+1)*64].T.astype(np.float64)
                c = qb_ @ kb_.T
                for i in range(M):
                    order = np.argsort(-c[i], kind="stable")
                    i1, i2 = int(order[0]), int(order[1])
                    col = p * 128 + i * 4 + hip * 2
                    selidx[0, col] = i1 * 64
                    selidx[0, col + 1] = i2 * 64
                    if i == i1 or i == i2:
                        wbias[hip*64:(hip+1)*64, p*32+i] = -30000.0
        in_maps.append({"xt": xt, "wq": wqkvT.astype(np.float16), "pw": pw,
                        "selidx": selidx, "wbias": wbias})
    return in_maps


def kernel(x, qkv_w, proj_w, proj_b):
    global _NC_CACHE, LAST_RESULTS
    x = np.asarray(x, np.float32)
    qkv_w = np.asarray(qkv_w, np.float32)
    proj_w = np.asarray(proj_w, np.float32)
    proj_b = np.asarray(proj_b, np.float32)

    if _NC_CACHE is None:
        _NC_CACHE = build_kernel()
    nc = _NC_CACHE

    in_maps = _host_prep(x, qkv_w, proj_w)
    res = run_bass_kernel_spmd(nc, in_maps, list(range(NCORES)))
    LAST_RESULTS = res

    out = np.zeros((B, N, DIM), np.float32)
    for core in range(NCORES):
        out[core // (NCORES // B)] += res.results[core]["y"].astype(np.float32)
    out += proj_b[None, None, :]
    return out


# revision 22
# speedup vs baseline: 1.1739x; 1.1349x over previous
"""Block-sparse attention (CABAttention) Trainium2 kernel — v3.

Sharding: 8 cores = 2 batches x 4 head-groups (4 heads each).
Per core: qkv projection (fp16 in, fp32 PSUM), top-2+diag block-sparse
attention (fp16 value path), output projection (row-parallel partial
sums, f16, host-summed + bias).

v3: PE-dynamic architecture (dynamic block offsets consumed as
register-offset moving operands on the tensor engine — the only
stable high-rate dynamic path: SP-issued dynamic DMAs cost ~730ns
sequencer time each, and DVE/ACT reg_loads hang the device). vs the
v1 baseline: ONE 4-register load per (pair, qblock) iteration instead
of four serialized single loads, fp16 qkv projection, anchor distance
6 (not 3), and the output projection interleaved into the attention
loop so the PE stays dense and the output DMA overlaps compute.
Block selection (top-2 of coarse block-mean scores) runs on host in
float64 and is passed as index inputs.
"""
import sys

sys.path.insert(0, "/opt/trn_rl_repo")

import numpy as np

import concourse.bass as bass
import concourse.mybir as mybir
import concourse.tile as tile
from concourse import bacc
from concourse.bass import ds
from concourse.bass_utils import run_bass_kernel_spmd
from concourse.masks import make_identity

F32 = mybir.dt.float32
F16 = mybir.dt.float16
I32 = mybir.dt.int32
ET = mybir.EngineType

DIM = 1024
H = 16
HD = 64
BS = 64
N = 2048
B = 2
M = N // BS            # 32 blocks
SCALE = HD ** -0.5
NCORES = 8
HPC = H // (NCORES // B)   # 4 heads per core

_NC_CACHE = None
LAST_RESULTS = None


def build_kernel():
    import os
    anchor_d = int(os.environ.get("ANCHOR_D", "8"))
    nc = bacc.Bacc(None)
    xt_d = nc.dram_tensor("xt", [DIM, N], F16, kind="ExternalInput")
    wq_d = nc.dram_tensor("wq", [DIM, 768], F16, kind="ExternalInput")
    pw_d = nc.dram_tensor("pw", [256, DIM], F16, kind="ExternalInput")
    idx_d = nc.dram_tensor("selidx", [1, 256], I32, kind="ExternalInput")
    wb_d = nc.dram_tensor("wbias", [128, 64], F32, kind="ExternalInput")
    y_d = nc.dram_tensor("y", [N, DIM], F16, kind="ExternalOutput")

    with tile.TileContext(nc) as tc:
        with tc.tile_pool(name="big", bufs=1) as big, \
             tc.tile_pool(name="wrk", bufs=8) as wrk:

            # ---- persistent SBUF tensors ----
            xt = big.tile([128, 8, N], F16)           # x^T, feature-major
            wq = big.tile([128, 8, 768], F16)         # qkv weights^T
            pwt = big.tile([128, 2, DIM], F16)        # proj weights
            idx = big.tile([1, 256], I32)
            wb = big.tile([128, 64], F32)
            qT = [big.tile([128, N], F16, name=f"qT{i}") for i in range(2)]
            kkT = [big.tile([128, N], F16, name=f"kkT{i}") for i in range(2)]
            vvT = [big.tile([128, N], F16, name=f"vvT{i}") for i in range(2)]
            vdAB = [big.tile([64, 2, N], F16, name=f"vdAB{i}")
                    for i in range(2)]
            vdA = [vdAB[i][:, 0, :] for i in range(2)]
            vdB = [vdAB[i][:, 1, :] for i in range(2)]
            outT = [big.tile([128, N], F16, name=f"outT{i}") for i in range(2)]
            qTB = [big.tile([64, N], F16, name=f"qTB{i}") for i in range(2)]
            kkTB = [big.tile([64, N], F16, name=f"kkTB{i}") for i in range(2)]
            identf = big.tile([128, 128], F32)
            ident = big.tile([128, 128], F16)

            # ---- input DMAs (split for pipelining) ----
            xt_v = xt_d[:].rearrange("(a p) n -> p a n", p=128)
            wq_v = wq_d[:].rearrange("(a p) n -> p a n", p=128)
            pw_v = pw_d[:].rearrange("(a p) n -> p a n", p=128)
            for k in range(8):
                nc.sync.dma_start(xt[:, k, :], xt_v[:, k, :])
                nc.sync.dma_start(wq[:, k, :], wq_v[:, k, :])
            nc.sync.dma_start(pwt[:], pw_v[:])
            idx_dma = nc.sync.dma_start(idx[:], idx_d[:])
            nc.sync.dma_start(wb[:], wb_d[:])

            make_identity(nc, identf[:])
            nc.vector.tensor_copy(ident[:], identf[:])

            # ---- qkv projection: fp16 inputs, fp32 PSUM over 8 K-chunks ----
            # M-tiles: 0,1 -> qT pair0/1; 2,4 -> kkT; 3,5 -> vvT
            tgt = [qT[0], qT[1], kkT[0], vvT[0], kkT[1], vvT[1]]
            with tc.tile_pool(name="qkps", bufs=4, space="PSUM") as qkps:
                for mt in range(6):
                    for nt in range(4):
                        ps = qkps.tile([128, 512], F32)
                        for k in range(8):
                            nc.tensor.matmul(
                                ps[:], lhsT=wq[:, k, mt * 128:(mt + 1) * 128],
                                rhs=xt[:, k, nt * 512:(nt + 1) * 512],
                                start=(k == 0), stop=(k == 7))
                        nc.vector.tensor_copy(
                            tgt[mt][:, nt * 512:(nt + 1) * 512], ps[:])

            # dynamic-offset matmul operands must sit at partition base 0:
            # make base-0 copies of the head-B halves
            for p in range(2):
                nc.sync.dma_start(qTB[p][:], qT[p][64:128, :])
                nc.sync.dma_start(kkTB[p][:], kkT[p][64:128, :])

            # ---- v_dup: per pair, transpose vvT blocks to keys-major ----
            with tc.tile_pool(name="vtps", bufs=2, space="PSUM") as vtps:
                for p in range(2):
                    for j in range(M):
                        tp = vtps.tile([64, 128], F16)
                        nc.tensor.transpose(
                            tp[:], vvT[p][:, j * 64:(j + 1) * 64], ident[:])
                        nc.vector.tensor_copy(
                            vdAB[p][:, 0, j * 64:(j + 1) * 64], tp[:, 0:64])
                        nc.vector.tensor_copy(
                            vdAB[p][:, 1, j * 64:(j + 1) * 64], tp[:, 64:128])

            # ---- block-sparse attention + interleaved projection ----
            # software-pipelined: stage A (scores+softmax), B (transpose
            # probs), C (AV), D (out transpose) of four consecutive
            # iterations interleave so the PE never waits on the
            # DVE/ACT softmax round-trips.
            with tc.tile_pool(name="fsps", bufs=4, space="PSUM") as fsps, \
                 tc.tile_pool(name="hsps", bufs=3, space="PSUM") as hsps, \
                 tc.tile_pool(name="ypsp", bufs=1, space="PSUM") as ypsp:
                anchors = {}
                st = {}
                NT = 2 * M

                def stage_a(t):
                    qb, p = t // 2, t % 2
                    qs = slice(qb * 64, (qb + 1) * 64)
                    base = p * 128 + qb * 4
                    loads, offs = nc.values_load_multi_w_load_instructions(
                        idx[0:1, base:base + 4], engines=[ET.PE],
                        min_val=0, max_val=N - 64,
                        skip_runtime_bounds_check=True)
                    for li in loads:
                        tile.add_dep_helper(li.ins, idx_dma.ins, sync=True,
                                            reason="idx dma -> pe regs")
                        if t >= anchor_d:
                            tile.add_dep_helper(
                                li.ins, anchors[t - anchor_d].ins, sync=False,
                                reason="bound PE register live range")
                    oA1, oA2, oB1, oB2 = offs
                    fs = fsps.tile([128, 320], F32, tag="fs")
                    sps = fs[:, 0:192]
                    for s, (oa, ob) in enumerate(
                            [(oA1, oB1), (oA2, oB2),
                             (qb * 64, qb * 64)]):
                        cs = slice(s * 64, (s + 1) * 64)
                        nc.tensor.matmul(
                            sps[0:64, cs], lhsT=qT[p][0:64, qs],
                            rhs=kkT[p][0:64, ds(oa, 64)],
                            start=True, stop=True)
                        nc.tensor.matmul(
                            sps[64:128, cs], lhsT=qTB[p][:, qs],
                            rhs=kkTB[p][:, ds(ob, 64)],
                            start=True, stop=True,
                            skip_group_check=True,
                            tile_position=(0, 64))
                    # mask duplicated diag slot (bias -30000 -> exp 0)
                    nc.scalar.activation(
                        sps[:, 128:192], sps[:, 128:192],
                        mybir.ActivationFunctionType.Identity,
                        bias=wb[:, p * 32 + qb:p * 32 + qb + 1])
                    pu = wrk.tile([128, 192], F16, tag="pu")
                    den = wrk.tile([128, 1], F32, tag="den")
                    nc.scalar.activation(pu[:], sps[:],
                                         mybir.ActivationFunctionType.Exp,
                                         accum_out=den[:])
                    rden = wrk.tile([128, 1], F32, tag="rden")
                    nc.vector.reciprocal(rden[:], den[:])
                    pr = wrk.tile([128, 192], F16, tag="pr")
                    nc.vector.tensor_scalar(pr[:], pu[:], rden[:, 0:1], None,
                                            op0=mybir.AluOpType.mult)
                    st[t] = {"offs": offs, "fs": fs, "pr": pr}

                def stage_b(t):
                    s_ = st[t]
                    hs = hsps.tile([128, 448], F16, tag="hs")
                    pt = hs[0:64, 0:384]
                    for s in range(3):
                        nc.tensor.transpose(
                            pt[:, s * 128:(s + 1) * 128],
                            s_["pr"][:, s * 64:(s + 1) * 64], ident[:])
                    pts = wrk.tile([64, 384], F16, tag="pts")
                    nc.vector.tensor_copy(pts[:], pt[:])
                    s_["hs"] = hs
                    s_["pts"] = pts

                def stage_c(t):
                    qb, p = t // 2, t % 2
                    s_ = st[t]
                    oA1, oA2, oB1, oB2 = s_["offs"]
                    pts = s_["pts"]
                    avpAB = s_["fs"][0:64, 192:320]
                    for s, o in enumerate([oA1, oA2, qb * 64]):
                        nc.tensor.matmul(
                            avpAB[:, 0:64],
                            lhsT=pts[:, s * 128:s * 128 + 64],
                            rhs=vdA[p][:, ds(o, 64)],
                            start=(s == 0), stop=(s == 2))
                    for s, o in enumerate([oB1, oB2, qb * 64]):
                        mi = nc.tensor.matmul(
                            avpAB[:, 64:128],
                            lhsT=pts[:, s * 128 + 64:s * 128 + 128],
                            rhs=vdB[p][:, ds(o, 64)],
                            start=(s == 0), stop=(s == 2))
                    anchors[t] = mi
                    av_sb = wrk.tile([64, 128], F16, tag="av_sb")
                    nc.scalar.copy(av_sb[:], avpAB[:])
                    s_["av_sb"] = av_sb

                def stage_d(t):
                    qb, p = t // 2, t % 2
                    qs = slice(qb * 64, (qb + 1) * 64)
                    s_ = st.pop(t)
                    otp = s_["hs"][:, 384:448]
                    nc.tensor.transpose(otp[:], s_["av_sb"][:],
                                        ident[0:64, 0:64])
                    nc.vector.tensor_copy(outT[p][:, qs], otp[:])

                def proj_half(tt, nt):
                    ts_ = slice(tt * 128, (tt + 1) * 128)
                    ns = slice(nt * 512, (nt + 1) * 512)
                    yp = ypsp.tile([128, 512], F32)
                    nc.tensor.matmul(yp[:], lhsT=outT[0][:, ts_],
                                     rhs=pwt[:, 0, ns],
                                     start=True, stop=False)
                    nc.tensor.matmul(yp[:], lhsT=outT[1][:, ts_],
                                     rhs=pwt[:, 1, ns],
                                     start=False, stop=True)
                    ys = wrk.tile([128, 512], F16, tag="ys")
                    nc.vector.tensor_copy(ys[:], yp[:])
                    nc.sync.dma_start(y_d[ts_, ns], ys[:])

                # iteration order: p inner-fast would split pairs; use
                # t = qb*2 + p so both pairs of a token tile finish on
                # consecutive t (proj after t = 4*tt+3 completes stage D)
                for w in range(NT + 3):
                    if w < NT:
                        stage_a(w)
                    if 0 <= w - 1 < NT:
                        stage_b(w - 1)
                    if 0 <= w - 2 < NT:
                        stage_c(w - 2)
                    if 0 <= w - 3 < NT:
                        td = w - 3
                        stage_d(td)
                        if td % 4 == 3:
                            proj_half(td // 4, 0)
                        elif td % 4 == 1 and td >= 5:
                            proj_half((td - 5) // 4, 1)
                proj_half(M // 2 - 1, 1)

    nc.finalize()
    return nc


def _host_prep(x, qkv_w, proj_w):
    """Per-core input maps + block selection (float64, matches fp32 ref)."""
    in_maps = []
    x64 = x.astype(np.float64)
    for core in range(NCORES):
        b = core // (NCORES // B)
        hg = core % (NCORES // B)
        heads = [hg * HPC + i for i in range(HPC)]

        xt = np.ascontiguousarray(x[b].T).astype(np.float16)

        wqkvT = np.empty((DIM, 768), np.float32)
        for p in range(2):
            hA, hB = heads[2 * p], heads[2 * p + 1]
            wqkvT[:, p*128:p*128+64] = qkv_w[hA*64:(hA+1)*64].T * SCALE
            wqkvT[:, p*128+64:p*128+128] = qkv_w[hB*64:(hB+1)*64].T * SCALE
            kbase = 256 + p * 256
            wqkvT[:, kbase:kbase+64] = qkv_w[DIM+hA*64:DIM+(hA+1)*64].T
            wqkvT[:, kbase+64:kbase+128] = qkv_w[DIM+hB*64:DIM+(hB+1)*64].T
            vbase = kbase + 128
            wqkvT[:, vbase:vbase+64] = qkv_w[2*DIM+hA*64:2*DIM+(hA+1)*64].T
            wqkvT[:, vbase+64:vbase+128] = qkv_w[2*DIM+hB*64:2*DIM+(hB+1)*64].T

        pw = np.ascontiguousarray(
            proj_w[:, heads[0]*64:(heads[-1]+1)*64].T).astype(np.float16)

        # float64 selection (matches fp32 reference ordering w/ margin)
        xb = x64[b].reshape(M, BS, DIM).mean(axis=1)
        selidx = np.zeros((1, 256), np.int32)
        wbias = np.zeros((128, 64), np.float32)
        for p in range(2):
            for hip in range(2):
                h = heads[2 * p + hip]
                qb_ = xb @ qkv_w[h*64:(h+1)*64].T.astype(np.float64)
                kb_ = xb @ qkv_w[DIM+h*64:DIM+(h+1)*64].T.astype(np.float64)
                c = qb_ @ kb_.T
                for i in range(M):
                    order = np.argsort(-c[i], kind="stable")
                    i1, i2 = int(order[0]), int(order[1])
                    col = p * 128 + i * 4 + hip * 2
                    selidx[0, col] = i1 * 64
                    selidx[0, col + 1] = i2 * 64
                    if i == i1 or i == i2:
                        wbias[hip*64:(hip+1)*64, p*32+i] = -30000.0
        in_maps.append({"xt": xt, "wq": wqkvT.astype(np.float16), "pw": pw,
                        "selidx": selidx, "wbias": wbias})
    return in_maps


def kernel(x, qkv_w, proj_w, proj_b):
    global _NC_CACHE, LAST_RESULTS
    x = np.asarray(x, np.float32)
    qkv_w = np.asarray(qkv_w, np.float32)
    proj_w = np.asarray(proj_w, np.float32)
    proj_b = np.asarray(proj_b, np.float32)

    if _NC_CACHE is None:
        _NC_CACHE = build_kernel()
    nc = _NC_CACHE

    in_maps = _host_prep(x, qkv_w, proj_w)
    res = run_bass_kernel_spmd(nc, in_maps, list(range(NCORES)))
    LAST_RESULTS = res

    out = np.zeros((B, N, DIM), np.float32)
    for core in range(NCORES):
        out[core // (NCORES // B)] += res.results[core]["y"].astype(np.float32)
    out += proj_b[None, None, :]
    return out


# revision 23
# speedup vs baseline: 1.2057x; 1.0271x over previous
"""Block-sparse attention (CABAttention) Trainium2 kernel — v3.

Sharding: 8 cores = 2 batches x 4 head-groups (4 heads each).
Per core: qkv projection (fp16 in, fp32 PSUM), top-2+diag block-sparse
attention (fp16 value path), output projection (row-parallel partial
sums, f16, host-summed + bias).

v3: PE-dynamic architecture (dynamic block offsets consumed as
register-offset moving operands on the tensor engine — the only
stable high-rate dynamic path: SP-issued dynamic DMAs cost ~730ns
sequencer time each, and DVE/ACT reg_loads hang the device). vs the
v1 baseline: ONE 4-register load per (pair, qblock) iteration instead
of four serialized single loads, fp16 qkv projection, anchor distance
6 (not 3), and the output projection interleaved into the attention
loop so the PE stays dense and the output DMA overlaps compute.
Block selection (top-2 of coarse block-mean scores) runs on host in
float64 and is passed as index inputs.
"""
import sys

sys.path.insert(0, "/opt/trn_rl_repo")

import numpy as np

import concourse.bass as bass
import concourse.mybir as mybir
import concourse.tile as tile
from concourse import bacc
from concourse.bass import ds
from concourse.bass_utils import run_bass_kernel_spmd
from concourse.masks import make_identity

F32 = mybir.dt.float32
F16 = mybir.dt.float16
I32 = mybir.dt.int32
ET = mybir.EngineType

DIM = 1024
H = 16
HD = 64
BS = 64
N = 2048
B = 2
M = N // BS            # 32 blocks
SCALE = HD ** -0.5
NCORES = 8
HPC = H // (NCORES // B)   # 4 heads per core

_NC_CACHE = None
LAST_RESULTS = None


def build_kernel():
    import os
    anchor_d = int(os.environ.get("ANCHOR_D", "8"))
    nc = bacc.Bacc(None)
    xt_d = nc.dram_tensor("xt", [DIM, N], F16, kind="ExternalInput")
    wq_d = nc.dram_tensor("wq", [DIM, 768], F16, kind="ExternalInput")
    pw_d = nc.dram_tensor("pw", [256, DIM], F16, kind="ExternalInput")
    idx_d = nc.dram_tensor("selidx", [1, 256], I32, kind="ExternalInput")
    wb_d = nc.dram_tensor("wbias", [128, 64], F32, kind="ExternalInput")
    y_d = nc.dram_tensor("y", [N, DIM], F16, kind="ExternalOutput")

    with tile.TileContext(nc) as tc:
        with tc.tile_pool(name="big", bufs=1) as big, \
             tc.tile_pool(name="wrk", bufs=8) as wrk:

            # ---- persistent SBUF tensors ----
            xt = big.tile([128, 8, N], F16)           # x^T, feature-major
            wq = big.tile([128, 8, 768], F16)         # qkv weights^T
            pwt = big.tile([128, 2, DIM], F16)        # proj weights
            idx = big.tile([1, 256], I32)
            wb = big.tile([128, 64], F32)
            qT = [big.tile([128, N], F16, name=f"qT{i}") for i in range(2)]
            kkT = [big.tile([128, N], F16, name=f"kkT{i}") for i in range(2)]
            vvT = [big.tile([128, N], F16, name=f"vvT{i}") for i in range(2)]
            vdAB = [big.tile([64, 2, N], F16, name=f"vdAB{i}")
                    for i in range(2)]
            vdA = [vdAB[i][:, 0, :] for i in range(2)]
            vdB = [vdAB[i][:, 1, :] for i in range(2)]
            outT = [big.tile([128, N], F16, name=f"outT{i}") for i in range(2)]
            qTB = [big.tile([64, N], F16, name=f"qTB{i}") for i in range(2)]
            kkTB = [big.tile([64, N], F16, name=f"kkTB{i}") for i in range(2)]
            identf = big.tile([128, 128], F32)
            ident = big.tile([128, 128], F16)

            # ---- input DMAs (split for pipelining) ----
            xt_v = xt_d[:].rearrange("(a p) n -> p a n", p=128)
            wq_v = wq_d[:].rearrange("(a p) n -> p a n", p=128)
            pw_v = pw_d[:].rearrange("(a p) n -> p a n", p=128)
            for k in range(8):
                nc.sync.dma_start(xt[:, k, :], xt_v[:, k, :])
                nc.sync.dma_start(wq[:, k, :], wq_v[:, k, :])
            nc.sync.dma_start(pwt[:], pw_v[:])
            idx_dma = nc.sync.dma_start(idx[:], idx_d[:])
            nc.sync.dma_start(wb[:], wb_d[:])

            make_identity(nc, identf[:])
            nc.vector.tensor_copy(ident[:], identf[:])

            # ---- qkv projection: fp16 inputs, fp32 PSUM over 8 K-chunks ----
            # M-tiles: 0,1 -> qT pair0/1; 2,4 -> kkT; 3,5 -> vvT
            tgt = [qT[0], qT[1], kkT[0], vvT[0], kkT[1], vvT[1]]
            with tc.tile_pool(name="qkps", bufs=8, space="PSUM") as qkps:
                for mt in range(6):
                    for nt in range(4):
                        ps = qkps.tile([128, 512], F32)
                        for k in range(8):
                            nc.tensor.matmul(
                                ps[:], lhsT=wq[:, k, mt * 128:(mt + 1) * 128],
                                rhs=xt[:, k, nt * 512:(nt + 1) * 512],
                                start=(k == 0), stop=(k == 7))
                        nc.vector.tensor_copy(
                            tgt[mt][:, nt * 512:(nt + 1) * 512], ps[:])

            # dynamic-offset matmul operands must sit at partition base 0:
            # make base-0 copies of the head-B halves
            for p in range(2):
                nc.sync.dma_start(qTB[p][:], qT[p][64:128, :])
                nc.sync.dma_start(kkTB[p][:], kkT[p][64:128, :])

            # ---- v_dup: per pair, transpose vvT blocks to keys-major ----
            with tc.tile_pool(name="vtps", bufs=2, space="PSUM") as vtps:
                for p in range(2):
                    for j in range(M):
                        tp = vtps.tile([64, 128], F16)
                        nc.tensor.transpose(
                            tp[:], vvT[p][:, j * 64:(j + 1) * 64], ident[:])
                        nc.vector.tensor_copy(
                            vdAB[p][:, 0, j * 64:(j + 1) * 64], tp[:, 0:64])
                        nc.vector.tensor_copy(
                            vdAB[p][:, 1, j * 64:(j + 1) * 64], tp[:, 64:128])

            # ---- block-sparse attention + interleaved projection ----
            # software-pipelined: stage A (scores+softmax), B (transpose
            # probs), C (AV), D (out transpose) of four consecutive
            # iterations interleave so the PE never waits on the
            # DVE/ACT softmax round-trips.
            with tc.tile_pool(name="fsps", bufs=4, space="PSUM") as fsps, \
                 tc.tile_pool(name="hsps", bufs=3, space="PSUM") as hsps, \
                 tc.tile_pool(name="ypsp", bufs=1, space="PSUM") as ypsp:
                anchors = {}
                st = {}
                NT = 2 * M

                def stage_a(t):
                    qb, p = t // 2, t % 2
                    qs = slice(qb * 64, (qb + 1) * 64)
                    base = p * 128 + qb * 4
                    loads, offs = nc.values_load_multi_w_load_instructions(
                        idx[0:1, base:base + 4], engines=[ET.PE],
                        min_val=0, max_val=N - 64,
                        skip_runtime_bounds_check=True)
                    for li in loads:
                        tile.add_dep_helper(li.ins, idx_dma.ins, sync=True,
                                            reason="idx dma -> pe regs")
                        if t >= anchor_d:
                            tile.add_dep_helper(
                                li.ins, anchors[t - anchor_d].ins, sync=False,
                                reason="bound PE register live range")
                    oA1, oA2, oB1, oB2 = offs
                    fs = fsps.tile([128, 320], F32, tag="fs")
                    sps = fs[:, 0:192]
                    for s, (oa, ob) in enumerate(
                            [(oA1, oB1), (oA2, oB2),
                             (qb * 64, qb * 64)]):
                        cs = slice(s * 64, (s + 1) * 64)
                        nc.tensor.matmul(
                            sps[0:64, cs], lhsT=qT[p][0:64, qs],
                            rhs=kkT[p][0:64, ds(oa, 64)],
                            start=True, stop=True)
                        nc.tensor.matmul(
                            sps[64:128, cs], lhsT=qTB[p][:, qs],
                            rhs=kkTB[p][:, ds(ob, 64)],
                            start=True, stop=True,
                            skip_group_check=True,
                            tile_position=(0, 64))
                    # mask duplicated diag slot (bias -30000 -> exp 0)
                    nc.scalar.activation(
                        sps[:, 128:192], sps[:, 128:192],
                        mybir.ActivationFunctionType.Identity,
                        bias=wb[:, p * 32 + qb:p * 32 + qb + 1])
                    pu = wrk.tile([128, 192], F16, tag="pu")
                    den = wrk.tile([128, 1], F32, tag="den")
                    nc.scalar.activation(pu[:], sps[:],
                                         mybir.ActivationFunctionType.Exp,
                                         accum_out=den[:])
                    rden = wrk.tile([128, 1], F32, tag="rden")
                    nc.vector.reciprocal(rden[:], den[:])
                    pr = wrk.tile([128, 192], F16, tag="pr")
                    nc.vector.tensor_scalar(pr[:], pu[:], rden[:, 0:1], None,
                                            op0=mybir.AluOpType.mult)
                    st[t] = {"offs": offs, "fs": fs, "pr": pr}

                def stage_b(t):
                    s_ = st[t]
                    hs = hsps.tile([128, 448], F16, tag="hs")
                    pt = hs[0:64, 0:384]
                    for s in range(3):
                        nc.tensor.transpose(
                            pt[:, s * 128:(s + 1) * 128],
                            s_["pr"][:, s * 64:(s + 1) * 64], ident[:])
                    pts = wrk.tile([64, 384], F16, tag="pts")
                    nc.vector.tensor_copy(pts[:], pt[:])
                    s_["hs"] = hs
                    s_["pts"] = pts

                def stage_c(t):
                    qb, p = t // 2, t % 2
                    s_ = st[t]
                    oA1, oA2, oB1, oB2 = s_["offs"]
                    pts = s_["pts"]
                    avpAB = s_["fs"][0:64, 192:320]
                    for s, o in enumerate([oA1, oA2, qb * 64]):
                        nc.tensor.matmul(
                            avpAB[:, 0:64],
                            lhsT=pts[:, s * 128:s * 128 + 64],
                            rhs=vdA[p][:, ds(o, 64)],
                            start=(s == 0), stop=(s == 2))
                    for s, o in enumerate([oB1, oB2, qb * 64]):
                        mi = nc.tensor.matmul(
                            avpAB[:, 64:128],
                            lhsT=pts[:, s * 128 + 64:s * 128 + 128],
                            rhs=vdB[p][:, ds(o, 64)],
                            start=(s == 0), stop=(s == 2))
                    anchors[t] = mi
                    av_sb = wrk.tile([64, 128], F16, tag="av_sb")
                    nc.scalar.copy(av_sb[:], avpAB[:])
                    s_["av_sb"] = av_sb

                def stage_d(t):
                    qb, p = t // 2, t % 2
                    qs = slice(qb * 64, (qb + 1) * 64)
                    s_ = st.pop(t)
                    otp = s_["hs"][:, 384:448]
                    nc.tensor.transpose(otp[:], s_["av_sb"][:],
                                        ident[0:64, 0:64])
                    nc.vector.tensor_copy(outT[p][:, qs], otp[:])

                def proj_half(tt, nt):
                    ts_ = slice(tt * 128, (tt + 1) * 128)
                    ns = slice(nt * 512, (nt + 1) * 512)
                    yp = ypsp.tile([128, 512], F32)
                    nc.tensor.matmul(yp[:], lhsT=outT[0][:, ts_],
                                     rhs=pwt[:, 0, ns],
                                     start=True, stop=False)
                    nc.tensor.matmul(yp[:], lhsT=outT[1][:, ts_],
                                     rhs=pwt[:, 1, ns],
                                     start=False, stop=True)
                    ys = wrk.tile([128, 512], F16, tag="ys")
                    nc.vector.tensor_copy(ys[:], yp[:])
                    nc.sync.dma_start(y_d[ts_, ns], ys[:])

                # iteration order: p inner-fast would split pairs; use
                # t = qb*2 + p so both pairs of a token tile finish on
                # consecutive t (proj after t = 4*tt+3 completes stage D)
                for w in range(NT + 3):
                    if w < NT:
                        stage_a(w)
                    if 0 <= w - 1 < NT:
                        stage_b(w - 1)
                    if 0 <= w - 2 < NT:
                        stage_c(w - 2)
                    if 0 <= w - 3 < NT:
                        td = w - 3
                        stage_d(td)
                        if td % 4 == 3:
                            proj_half(td // 4, 0)
                        elif td % 4 == 1 and td >= 5:
                            proj_half((td - 5) // 4, 1)
                proj_half(M // 2 - 1, 1)

    nc.finalize()
    return nc


def _host_prep(x, qkv_w, proj_w):
    """Per-core input maps + block selection (float64, matches fp32 ref)."""
    in_maps = []
    x64 = x.astype(np.float64)
    for core in range(NCORES):
        b = core // (NCORES // B)
        hg = core % (NCORES // B)
        heads = [hg * HPC + i for i in range(HPC)]

        xt = np.ascontiguousarray(x[b].T).astype(np.float16)

        wqkvT = np.empty((DIM, 768), np.float32)
        for p in range(2):
            hA, hB = heads[2 * p], heads[2 * p + 1]
            wqkvT[:, p*128:p*128+64] = qkv_w[hA*64:(hA+1)*64].T * SCALE
            wqkvT[:, p*128+64:p*128+128] = qkv_w[hB*64:(hB+1)*64].T * SCALE
            kbase = 256 + p * 256
            wqkvT[:, kbase:kbase+64] = qkv_w[DIM+hA*64:DIM+(hA+1)*64].T
            wqkvT[:, kbase+64:kbase+128] = qkv_w[DIM+hB*64:DIM+(hB+1)*64].T
            vbase = kbase + 128
            wqkvT[:, vbase:vbase+64] = qkv_w[2*DIM+hA*64:2*DIM+(hA+1)*64].T
            wqkvT[:, vbase+64:vbase+128] = qkv_w[2*DIM+hB*64:2*DIM+(hB+1)*64].T

        pw = np.ascontiguousarray(
            proj_w[:, heads[0]*64:(heads[-1]+1)*64].T).astype(np.float16)

        # float64 selection (matches fp32 reference ordering w/ margin)
        xb = x64[b].reshape(M, BS, DIM).mean(axis=1)
        selidx = np.zeros((1, 256), np.int32)
        wbias = np.zeros((128, 64), np.float32)
        for p in range(2):
            for hip in range(2):
                h = heads[2 * p + hip]
                qb_ = xb @ qkv_w[h*64:(h+1)*64].T.astype(np.float64)
                kb_ = xb @ qkv_w[DIM+h*64:DIM+(h+1)*64].T.astype(np.float64)
                c = qb_ @ kb_.T
                for i in range(M):
                    order = np.argsort(-c[i], kind="stable")
                    i1, i2 = int(order[0]), int(order[1])
                    col = p * 128 + i * 4 + hip * 2
                    selidx[0, col] = i1 * 64
                    selidx[0, col + 1] = i2 * 64
                    if i == i1 or i == i2:
                        wbias[hip*64:(hip+1)*64, p*32+i] = -30000.0
        in_maps.append({"xt": xt, "wq": wqkvT.astype(np.float16), "pw": pw,
                        "selidx": selidx, "wbias": wbias})
    return in_maps


def kernel(x, qkv_w, proj_w, proj_b):
    global _NC_CACHE, LAST_RESULTS
    x = np.asarray(x, np.float32)
    qkv_w = np.asarray(qkv_w, np.float32)
    proj_w = np.asarray(proj_w, np.float32)
    proj_b = np.asarray(proj_b, np.float32)

    if _NC_CACHE is None:
        _NC_CACHE = build_kernel()
    nc = _NC_CACHE

    in_maps = _host_prep(x, qkv_w, proj_w)
    res = run_bass_kernel_spmd(nc, in_maps, list(range(NCORES)))
    LAST_RESULTS = res

    out = np.zeros((B, N, DIM), np.float32)
    for core in range(NCORES):
        out[core // (NCORES // B)] += res.results[core]["y"].astype(np.float32)
    out += proj_b[None, None, :]
    return out


# revision 26
# speedup vs baseline: 1.2731x; 1.0559x over previous
"""Block-sparse attention (CABAttention) Trainium2 kernel — v3.

Sharding: 8 cores = 2 batches x 4 head-groups (4 heads each).
Per core: qkv projection (fp16 in, fp32 PSUM), top-2+diag block-sparse
attention (fp16 value path), output projection (row-parallel partial
sums, f16, host-summed + bias).

v3: PE-dynamic architecture (dynamic block offsets consumed as
register-offset moving operands on the tensor engine — the only
stable high-rate dynamic path: SP-issued dynamic DMAs cost ~730ns
sequencer time each, and DVE/ACT reg_loads hang the device). vs the
v1 baseline: ONE 4-register load per (pair, qblock) iteration instead
of four serialized single loads, fp16 qkv projection, anchor distance
6 (not 3), and the output projection interleaved into the attention
loop so the PE stays dense and the output DMA overlaps compute.
Block selection (top-2 of coarse block-mean scores) runs on host in
float64 and is passed as index inputs.
"""
import sys

sys.path.insert(0, "/opt/trn_rl_repo")

import numpy as np

import concourse.bass as bass
import concourse.mybir as mybir
import concourse.tile as tile
from concourse import bacc
from concourse.bass import ds
from concourse.bass_utils import run_bass_kernel_spmd
from concourse.masks import make_identity

F32 = mybir.dt.float32
F16 = mybir.dt.float16
I32 = mybir.dt.int32
ET = mybir.EngineType

DIM = 1024
H = 16
HD = 64
BS = 64
N = 2048
B = 2
M = N // BS            # 32 blocks
SCALE = HD ** -0.5
NCORES = 8
HPC = H // (NCORES // B)   # 4 heads per core

_NC_CACHE = None
LAST_RESULTS = None


def build_kernel():
    import os
    anchor_d = int(os.environ.get("ANCHOR_D", "8"))
    nc = bacc.Bacc(None)
    xt_d = nc.dram_tensor("xt", [DIM, N], F16, kind="ExternalInput")
    wq_d = nc.dram_tensor("wq", [DIM, 768], F16, kind="ExternalInput")
    pw_d = nc.dram_tensor("pw", [256, DIM], F16, kind="ExternalInput")
    idx_d = nc.dram_tensor("selidx", [1, 256], I32, kind="ExternalInput")
    wb_d = nc.dram_tensor("wbias", [128, 64], F32, kind="ExternalInput")
    y_d = nc.dram_tensor("y", [N, DIM], F16, kind="ExternalOutput")

    with tile.TileContext(nc) as tc:
        with tc.tile_pool(name="big", bufs=1) as big, \
             tc.tile_pool(name="wrk", bufs=8) as wrk:

            # ---- persistent SBUF tensors ----
            xt = big.tile([128, 8, N], F16)           # x^T, feature-major
            wq = big.tile([128, 8, 768], F16)         # qkv weights^T
            pwt = big.tile([128, 2, DIM], F16)        # proj weights
            idx = big.tile([1, 256], I32)
            wb = big.tile([128, 64], F32)
            qT = [big.tile([128, N], F16, name=f"qT{i}") for i in range(2)]
            kkT = [big.tile([128, N], F16, name=f"kkT{i}") for i in range(2)]
            vvT = [big.tile([128, N], F16, name=f"vvT{i}") for i in range(2)]
            vdAB = [big.tile([64, 2, N], F16, name=f"vdAB{i}")
                    for i in range(2)]
            vdA = [vdAB[i][:, 0, :] for i in range(2)]
            vdB = [vdAB[i][:, 1, :] for i in range(2)]
            outT = [big.tile([128, N], F16, name=f"outT{i}") for i in range(2)]
            qTB = [big.tile([64, N], F16, name=f"qTB{i}") for i in range(2)]
            kkTB = [big.tile([64, N], F16, name=f"kkTB{i}") for i in range(2)]
            identf = big.tile([128, 128], F32)
            ident = big.tile([128, 128], F16)

            # ---- input DMAs (split for pipelining) ----
            xt_v = xt_d[:].rearrange("(a p) n -> p a n", p=128)
            wq_v = wq_d[:].rearrange("(a p) n -> p a n", p=128)
            pw_v = pw_d[:].rearrange("(a p) n -> p a n", p=128)
            for k in range(8):
                nc.sync.dma_start(xt[:, k, :], xt_v[:, k, :])
                nc.sync.dma_start(wq[:, k, :], wq_v[:, k, :])
            nc.sync.dma_start(pwt[:], pw_v[:])
            idx_dma = nc.sync.dma_start(idx[:], idx_d[:])
            nc.sync.dma_start(wb[:], wb_d[:])

            make_identity(nc, identf[:])
            nc.vector.tensor_copy(ident[:], identf[:])

            # ---- qkv projection: fp16 inputs, fp32 PSUM over 8 K-chunks ----
            # m-tile order: K first (attention needs it first), V next
            # (v_dup interleaves right after each pair's V tile), Q last
            # (its base-0 dup is cheap); head-B base-0 dup DMAs issue as
            # soon as their source is complete.
            tgt = [qT[0], qT[1], kkT[0], vvT[0], kkT[1], vvT[1]]
            with tc.tile_pool(name="qkps", bufs=6, space="PSUM") as qkps, \
                 tc.tile_pool(name="vtps", bufs=2, space="PSUM") as vtps:

                def mtile(mt):
                    for nt in range(4):
                        ps = qkps.tile([128, 512], F32)
                        for k in range(8):
                            nc.tensor.matmul(
                                ps[:], lhsT=wq[:, k, mt * 128:(mt + 1) * 128],
                                rhs=xt[:, k, nt * 512:(nt + 1) * 512],
                                start=(k == 0), stop=(k == 7))
                        nc.vector.tensor_copy(
                            tgt[mt][:, nt * 512:(nt + 1) * 512], ps[:])

                def vdup(p):
                    for j in range(M):
                        tp = vtps.tile([64, 128], F16)
                        nc.tensor.transpose(
                            tp[:], vvT[p][:, j * 64:(j + 1) * 64], ident[:])
                        nc.vector.tensor_copy(
                            vdAB[p][:, 0, j * 64:(j + 1) * 64], tp[:, 0:64])
                        nc.vector.tensor_copy(
                            vdAB[p][:, 1, j * 64:(j + 1) * 64], tp[:, 64:128])

                mtile(2)
                nc.sync.dma_start(kkTB[0][:], kkT[0][64:128, :])
                mtile(4)
                nc.sync.dma_start(kkTB[1][:], kkT[1][64:128, :])
                mtile(3)
                vdup(0)
                mtile(5)
                vdup(1)
                mtile(0)
                nc.sync.dma_start(qTB[0][:], qT[0][64:128, :])
                mtile(1)
                nc.sync.dma_start(qTB[1][:], qT[1][64:128, :])

            # ---- block-sparse attention + interleaved projection ----
            # software-pipelined: stage A (scores+softmax), B (transpose
            # probs), C (AV), D (out transpose) of four consecutive
            # iterations interleave so the PE never waits on the
            # DVE/ACT softmax round-trips.
            with tc.tile_pool(name="fsps", bufs=4, space="PSUM") as fsps, \
                 tc.tile_pool(name="hsps", bufs=3, space="PSUM") as hsps, \
                 tc.tile_pool(name="ypsp", bufs=1, space="PSUM") as ypsp:
                anchors = {}
                st = {}
                NT = 2 * M

                def stage_a(t):
                    qb, p = t // 2, t % 2
                    qs = slice(qb * 64, (qb + 1) * 64)
                    base = p * 128 + qb * 4
                    loads, offs = nc.values_load_multi_w_load_instructions(
                        idx[0:1, base:base + 4], engines=[ET.PE],
                        min_val=0, max_val=N - 64,
                        skip_runtime_bounds_check=True)
                    for li in loads:
                        tile.add_dep_helper(li.ins, idx_dma.ins, sync=True,
                                            reason="idx dma -> pe regs")
                        if t >= anchor_d:
                            tile.add_dep_helper(
                                li.ins, anchors[t - anchor_d].ins, sync=False,
                                reason="bound PE register live range")
                    oA1, oA2, oB1, oB2 = offs
                    fs = fsps.tile([128, 320], F32, tag="fs")
                    sps = fs[:, 0:192]
                    for s, (oa, ob) in enumerate(
                            [(oA1, oB1), (oA2, oB2),
                             (qb * 64, qb * 64)]):
                        cs = slice(s * 64, (s + 1) * 64)
                        nc.tensor.matmul(
                            sps[0:64, cs], lhsT=qT[p][0:64, qs],
                            rhs=kkT[p][0:64, ds(oa, 64)],
                            start=True, stop=True)
                        nc.tensor.matmul(
                            sps[64:128, cs], lhsT=qTB[p][:, qs],
                            rhs=kkTB[p][:, ds(ob, 64)],
                            start=True, stop=True,
                            skip_group_check=True,
                            tile_position=(0, 64))
                    # mask duplicated diag slot (bias -30000 -> exp 0)
                    nc.scalar.activation(
                        sps[:, 128:192], sps[:, 128:192],
                        mybir.ActivationFunctionType.Identity,
                        bias=wb[:, p * 32 + qb:p * 32 + qb + 1])
                    pu = wrk.tile([128, 192], F16, tag="pu")
                    den = wrk.tile([128, 1], F32, tag="den")
                    nc.scalar.activation(pu[:], sps[:],
                                         mybir.ActivationFunctionType.Exp,
                                         accum_out=den[:])
                    rden = wrk.tile([128, 1], F32, tag="rden")
                    nc.vector.reciprocal(rden[:], den[:])
                    pr = wrk.tile([128, 192], F16, tag="pr")
                    nc.vector.tensor_scalar(pr[:], pu[:], rden[:, 0:1], None,
                                            op0=mybir.AluOpType.mult)
                    st[t] = {"offs": offs, "fs": fs, "pr": pr}

                def stage_b(t):
                    s_ = st[t]
                    hs = hsps.tile([128, 448], F16, tag="hs")
                    pt = hs[0:64, 0:384]
                    for s in range(3):
                        nc.tensor.transpose(
                            pt[:, s * 128:(s + 1) * 128],
                            s_["pr"][:, s * 64:(s + 1) * 64], ident[:])
                    pts = wrk.tile([64, 384], F16, tag="pts")
                    nc.vector.tensor_copy(pts[:], pt[:])
                    s_["hs"] = hs
                    s_["pts"] = pts

                def stage_c(t):
                    qb, p = t // 2, t % 2
                    s_ = st[t]
                    oA1, oA2, oB1, oB2 = s_["offs"]
                    pts = s_["pts"]
                    avpAB = s_["fs"][0:64, 192:320]
                    for s, o in enumerate([oA1, oA2, qb * 64]):
                        nc.tensor.matmul(
                            avpAB[:, 0:64],
                            lhsT=pts[:, s * 128:s * 128 + 64],
                            rhs=vdA[p][:, ds(o, 64)],
                            start=(s == 0), stop=(s == 2))
                    for s, o in enumerate([oB1, oB2, qb * 64]):
                        mi = nc.tensor.matmul(
                            avpAB[:, 64:128],
                            lhsT=pts[:, s * 128 + 64:s * 128 + 128],
                            rhs=vdB[p][:, ds(o, 64)],
                            start=(s == 0), stop=(s == 2))
                    anchors[t] = mi
                    av_sb = wrk.tile([64, 128], F16, tag="av_sb")
                    nc.scalar.copy(av_sb[:], avpAB[:])
                    s_["av_sb"] = av_sb

                def stage_d(t):
                    qb, p = t // 2, t % 2
                    qs = slice(qb * 64, (qb + 1) * 64)
                    s_ = st.pop(t)
                    otp = s_["hs"][:, 384:448]
                    nc.tensor.transpose(otp[:], s_["av_sb"][:],
                                        ident[0:64, 0:64])
                    nc.vector.tensor_copy(outT[p][:, qs], otp[:])

                def proj_half(tt, nt):
                    ts_ = slice(tt * 128, (tt + 1) * 128)
                    ns = slice(nt * 512, (nt + 1) * 512)
                    yp = ypsp.tile([128, 512], F32)
                    nc.tensor.matmul(yp[:], lhsT=outT[0][:, ts_],
                                     rhs=pwt[:, 0, ns],
                                     start=True, stop=False)
                    nc.tensor.matmul(yp[:], lhsT=outT[1][:, ts_],
                                     rhs=pwt[:, 1, ns],
                                     start=False, stop=True)
                    ys = wrk.tile([128, 512], F16, tag="ys")
                    nc.vector.tensor_copy(ys[:], yp[:])
                    nc.sync.dma_start(y_d[ts_, ns], ys[:])

                # iteration order: p inner-fast would split pairs; use
                # t = qb*2 + p so both pairs of a token tile finish on
                # consecutive t (proj after t = 4*tt+3 completes stage D)
                for w in range(NT + 3):
                    if w < NT:
                        stage_a(w)
                    if 0 <= w - 1 < NT:
                        stage_b(w - 1)
                    if 0 <= w - 2 < NT:
                        stage_c(w - 2)
                    if 0 <= w - 3 < NT:
                        td = w - 3
                        stage_d(td)
                        if td % 4 == 3:
                            proj_half(td // 4, 0)
                        elif td % 4 == 1 and td >= 5:
                            proj_half((td - 5) // 4, 1)
                proj_half(M // 2 - 1, 1)

    nc.finalize()
    return nc


def _host_prep(x, qkv_w, proj_w):
    """Per-core input maps + block selection (float64, matches fp32 ref)."""
    in_maps = []
    x64 = x.astype(np.float64)
    for core in range(NCORES):
        b = core // (NCORES // B)
        hg = core % (NCORES // B)
        heads = [hg * HPC + i for i in range(HPC)]

        xt = np.ascontiguousarray(x[b].T).astype(np.float16)

        wqkvT = np.empty((DIM, 768), np.float32)
        for p in range(2):
            hA, hB = heads[2 * p], heads[2 * p + 1]
            wqkvT[:, p*128:p*128+64] = qkv_w[hA*64:(hA+1)*64].T * SCALE
            wqkvT[:, p*128+64:p*128+128] = qkv_w[hB*64:(hB+1)*64].T * SCALE
            kbase = 256 + p * 256
            wqkvT[:, kbase:kbase+64] = qkv_w[DIM+hA*64:DIM+(hA+1)*64].T
            wqkvT[:, kbase+64:kbase+128] = qkv_w[DIM+hB*64:DIM+(hB+1)*64].T
            vbase = kbase + 128
            wqkvT[:, vbase:vbase+64] = qkv_w[2*DIM+hA*64:2*DIM+(hA+1)*64].T
            wqkvT[:, vbase+64:vbase+128] = qkv_w[2*DIM+hB*64:2*DIM+(hB+1)*64].T

        pw = np.ascontiguousarray(
            proj_w[:, heads[0]*64:(heads[-1]+1)*64].T).astype(np.float16)

        # float64 selection (matches fp32 reference ordering w/ margin)
        xb = x64[b].reshape(M, BS, DIM).mean(axis=1)
        selidx = np.zeros((1, 256), np.int32)
        wbias = np.zeros((128, 64), np.float32)
        for p in range(2):
            for hip in range(2):
                h = heads[2 * p + hip]
                qb_ = xb @ qkv_w[h*64:(h+1)*64].T.astype(np.float64)
                kb_ = xb @ qkv_w[DIM+h*64:DIM+(h+1)*64].T.astype(np.float64)
                c = qb_ @ kb_.T
                for i in range(M):
                    order = np.argsort(-c[i], kind="stable")
                    i1, i2 = int(order[0]), int(order[1])
                    col = p * 128 + i * 4 + hip * 2
                    selidx[0, col] = i1 * 64
                    selidx[0, col + 1] = i2 * 64
                    if i == i1 or i == i2:
                        wbias[hip*64:(hip+1)*64, p*32+i] = -30000.0
        in_maps.append({"xt": xt, "wq": wqkvT.astype(np.float16), "pw": pw,
                        "selidx": selidx, "wbias": wbias})
    return in_maps


def kernel(x, qkv_w, proj_w, proj_b):
    global _NC_CACHE, LAST_RESULTS
    x = np.asarray(x, np.float32)
    qkv_w = np.asarray(qkv_w, np.float32)
    proj_w = np.asarray(proj_w, np.float32)
    proj_b = np.asarray(proj_b, np.float32)

    if _NC_CACHE is None:
        _NC_CACHE = build_kernel()
    nc = _NC_CACHE

    in_maps = _host_prep(x, qkv_w, proj_w)
    res = run_bass_kernel_spmd(nc, in_maps, list(range(NCORES)))
    LAST_RESULTS = res

    out = np.zeros((B, N, DIM), np.float32)
    for core in range(NCORES):
        out[core // (NCORES // B)] += res.results[core]["y"].astype(np.float32)
    out += proj_b[None, None, :]
    return out


# revision 28
# speedup vs baseline: 1.2745x; 1.0011x over previous
"""Block-sparse attention (CABAttention) Trainium2 kernel — v3.

Sharding: 8 cores = 2 batches x 4 head-groups (4 heads each).
Per core: qkv projection (fp16 in, fp32 PSUM), top-2+diag block-sparse
attention (fp16 value path), output projection (row-parallel partial
sums, f16, host-summed + bias).

v3: PE-dynamic architecture (dynamic block offsets consumed as
register-offset moving operands on the tensor engine — the only
stable high-rate dynamic path: SP-issued dynamic DMAs cost ~730ns
sequencer time each, and DVE/ACT reg_loads hang the device). vs the
v1 baseline: ONE 4-register load per (pair, qblock) iteration instead
of four serialized single loads, fp16 qkv projection, anchor distance
6 (not 3), and the output projection interleaved into the attention
loop so the PE stays dense and the output DMA overlaps compute.
Block selection (top-2 of coarse block-mean scores) runs on host in
float64 and is passed as index inputs.
"""
import sys

sys.path.insert(0, "/opt/trn_rl_repo")

import numpy as np

import concourse.bass as bass
import concourse.mybir as mybir
import concourse.tile as tile
from concourse import bacc
from concourse.bass import ds
from concourse.bass_utils import run_bass_kernel_spmd
from concourse.masks import make_identity

F32 = mybir.dt.float32
F16 = mybir.dt.float16
I32 = mybir.dt.int32
ET = mybir.EngineType

DIM = 1024
H = 16
HD = 64
BS = 64
N = 2048
B = 2
M = N // BS            # 32 blocks
SCALE = HD ** -0.5
NCORES = 8
HPC = H // (NCORES // B)   # 4 heads per core

_NC_CACHE = None
LAST_RESULTS = None


def build_kernel():
    import os
    anchor_d = int(os.environ.get("ANCHOR_D", "8"))
    nc = bacc.Bacc(None)
    xt_d = nc.dram_tensor("xt", [DIM, N], F16, kind="ExternalInput")
    wq_d = nc.dram_tensor("wq", [DIM, 768], F16, kind="ExternalInput")
    pw_d = nc.dram_tensor("pw", [256, DIM], F16, kind="ExternalInput")
    idx_d = nc.dram_tensor("selidx", [1, 256], I32, kind="ExternalInput")
    wb_d = nc.dram_tensor("wbias", [128, 64], F32, kind="ExternalInput")
    y_d = nc.dram_tensor("y", [N, DIM], F16, kind="ExternalOutput")

    with tile.TileContext(nc) as tc:
        with tc.tile_pool(name="big", bufs=1) as big, \
             tc.tile_pool(name="wrk", bufs=8) as wrk:

            # ---- persistent SBUF tensors ----
            xt = big.tile([128, 8, N], F16)           # x^T, feature-major
            wq = big.tile([128, 8, 768], F16)         # qkv weights^T
            pwt = big.tile([128, 2, DIM], F16)        # proj weights
            idx = big.tile([1, 256], I32)
            wb = big.tile([128, 64], F32)
            qT = [big.tile([128, N], F16, name=f"qT{i}") for i in range(2)]
            kkT = [big.tile([128, N], F16, name=f"kkT{i}") for i in range(2)]
            vvT = [big.tile([128, N], F16, name=f"vvT{i}") for i in range(2)]
            vdAB = [big.tile([64, 2, N], F16, name=f"vdAB{i}")
                    for i in range(2)]
            vdA = [vdAB[i][:, 0, :] for i in range(2)]
            vdB = [vdAB[i][:, 1, :] for i in range(2)]
            outT = [big.tile([128, N], F16, name=f"outT{i}") for i in range(2)]
            qTB = [big.tile([64, N], F16, name=f"qTB{i}") for i in range(2)]
            kkTB = [big.tile([64, N], F16, name=f"kkTB{i}") for i in range(2)]
            identf = big.tile([128, 128], F32)
            ident = big.tile([128, 128], F16)

            # ---- input DMAs (split for pipelining) ----
            xt_v = xt_d[:].rearrange("(a p) n -> p a n", p=128)
            wq_v = wq_d[:].rearrange("(a p) n -> p a n", p=128)
            pw_v = pw_d[:].rearrange("(a p) n -> p a n", p=128)
            for k in range(8):
                nc.sync.dma_start(xt[:, k, :], xt_v[:, k, :])
                nc.sync.dma_start(wq[:, k, :], wq_v[:, k, :])
            nc.sync.dma_start(pwt[:], pw_v[:])
            idx_dma = nc.sync.dma_start(idx[:], idx_d[:])
            nc.sync.dma_start(wb[:], wb_d[:])

            make_identity(nc, identf[:])
            nc.vector.tensor_copy(ident[:], identf[:])

            # ---- qkv projection: fp16 inputs, fp32 PSUM over 8 K-chunks ----
            # m-tile order: K first (attention needs it first), V next
            # (v_dup interleaves right after each pair's V tile), Q last
            # (its base-0 dup is cheap); head-B base-0 dup DMAs issue as
            # soon as their source is complete.
            tgt = [qT[0], qT[1], kkT[0], vvT[0], kkT[1], vvT[1]]
            with tc.tile_pool(name="qkps", bufs=6, space="PSUM") as qkps, \
                 tc.tile_pool(name="vtps", bufs=2, space="PSUM") as vtps:

                def mtile(mt):
                    for nt in range(4):
                        ps = qkps.tile([128, 512], F32)
                        for k in range(8):
                            nc.tensor.matmul(
                                ps[:], lhsT=wq[:, k, mt * 128:(mt + 1) * 128],
                                rhs=xt[:, k, nt * 512:(nt + 1) * 512],
                                start=(k == 0), stop=(k == 7))
                        nc.vector.tensor_copy(
                            tgt[mt][:, nt * 512:(nt + 1) * 512], ps[:])

                def vdup(p):
                    for j in range(M):
                        tp = vtps.tile([64, 128], F16)
                        nc.tensor.transpose(
                            tp[:], vvT[p][:, j * 64:(j + 1) * 64], ident[:])
                        nc.vector.tensor_copy(
                            vdAB[p][:, 0, j * 64:(j + 1) * 64], tp[:, 0:64])
                        nc.vector.tensor_copy(
                            vdAB[p][:, 1, j * 64:(j + 1) * 64], tp[:, 64:128])

                mtile(2)
                nc.sync.dma_start(kkTB[0][:], kkT[0][64:128, :])
                mtile(4)
                nc.sync.dma_start(kkTB[1][:], kkT[1][64:128, :])
                mtile(3)
                vdup(0)
                mtile(5)
                vdup(1)
                mtile(0)
                nc.sync.dma_start(qTB[0][:], qT[0][64:128, :])
                mtile(1)
                nc.sync.dma_start(qTB[1][:], qT[1][64:128, :])

            # ---- block-sparse attention + interleaved projection ----
            # software-pipelined: stage A (scores+softmax), B (transpose
            # probs), C (AV), D (out transpose) of four consecutive
            # iterations interleave so the PE never waits on the
            # DVE/ACT softmax round-trips.
            with tc.tile_pool(name="fsps", bufs=4, space="PSUM") as fsps, \
                 tc.tile_pool(name="hsps", bufs=3, space="PSUM") as hsps, \
                 tc.tile_pool(name="ypsp", bufs=1, space="PSUM") as ypsp:
                anchors = {}
                st = {}
                NT = 2 * M

                def stage_a(t):
                    qb, p = t // 2, t % 2
                    qs = slice(qb * 64, (qb + 1) * 64)
                    base = p * 128 + qb * 4
                    loads, offs = nc.values_load_multi_w_load_instructions(
                        idx[0:1, base:base + 4], engines=[ET.PE],
                        min_val=0, max_val=N - 64,
                        skip_runtime_bounds_check=True)
                    for li in loads:
                        tile.add_dep_helper(li.ins, idx_dma.ins, sync=True,
                                            reason="idx dma -> pe regs")
                        if t >= anchor_d:
                            tile.add_dep_helper(
                                li.ins, anchors[t - anchor_d].ins, sync=False,
                                reason="bound PE register live range")
                    oA1, oA2, oB1, oB2 = offs
                    fs = fsps.tile([128, 320], F32, tag="fs")
                    sps = fs[:, 0:192]
                    for s, (oa, ob) in enumerate(
                            [(oA1, oB1), (oA2, oB2),
                             (qb * 64, qb * 64)]):
                        cs = slice(s * 64, (s + 1) * 64)
                        nc.tensor.matmul(
                            sps[0:64, cs], lhsT=qT[p][0:64, qs],
                            rhs=kkT[p][0:64, ds(oa, 64)],
                            start=True, stop=True)
                        nc.tensor.matmul(
                            sps[64:128, cs], lhsT=qTB[p][:, qs],
                            rhs=kkTB[p][:, ds(ob, 64)],
                            start=True, stop=True,
                            skip_group_check=True,
                            tile_position=(0, 64))
                    # mask duplicated diag slot (bias -30000 -> exp 0)
                    nc.scalar.activation(
                        sps[:, 128:192], sps[:, 128:192],
                        mybir.ActivationFunctionType.Identity,
                        bias=wb[:, p * 32 + qb:p * 32 + qb + 1])
                    pu = wrk.tile([128, 192], F16, tag="pu")
                    den = wrk.tile([128, 1], F32, tag="den")
                    nc.scalar.activation(pu[:], sps[:],
                                         mybir.ActivationFunctionType.Exp,
                                         accum_out=den[:])
                    rden = wrk.tile([128, 1], F32, tag="rden")
                    nc.vector.reciprocal(rden[:], den[:])
                    pr = wrk.tile([128, 192], F16, tag="pr")
                    nc.vector.tensor_scalar(pr[:], pu[:], rden[:, 0:1], None,
                                            op0=mybir.AluOpType.mult)
                    st[t] = {"offs": offs, "fs": fs, "pr": pr}

                def stage_b(t):
                    s_ = st[t]
                    hs = hsps.tile([128, 448], F16, tag="hs")
                    pt = hs[0:64, 0:384]
                    for s in range(3):
                        nc.tensor.transpose(
                            pt[:, s * 128:(s + 1) * 128],
                            s_["pr"][:, s * 64:(s + 1) * 64], ident[:])
                    pts = wrk.tile([64, 384], F16, tag="pts")
                    nc.vector.tensor_copy(pts[:], pt[:])
                    s_["hs"] = hs
                    s_["pts"] = pts

                def stage_c(t):
                    qb, p = t // 2, t % 2
                    s_ = st[t]
                    oA1, oA2, oB1, oB2 = s_["offs"]
                    pts = s_["pts"]
                    avpAB = s_["fs"][0:64, 192:320]
                    for s, o in enumerate([oA1, oA2, qb * 64]):
                        nc.tensor.matmul(
                            avpAB[:, 0:64],
                            lhsT=pts[:, s * 128:s * 128 + 64],
                            rhs=vdA[p][:, ds(o, 64)],
                            start=(s == 0), stop=(s == 2))
                    for s, o in enumerate([oB1, oB2, qb * 64]):
                        mi = nc.tensor.matmul(
                            avpAB[:, 64:128],
                            lhsT=pts[:, s * 128 + 64:s * 128 + 128],
                            rhs=vdB[p][:, ds(o, 64)],
                            start=(s == 0), stop=(s == 2))
                    anchors[t] = mi
                    av_sb = wrk.tile([64, 128], F16, tag="av_sb")
                    nc.scalar.copy(av_sb[:], avpAB[:])
                    s_["av_sb"] = av_sb

                def stage_d(t):
                    qb, p = t // 2, t % 2
                    qs = slice(qb * 64, (qb + 1) * 64)
                    s_ = st.pop(t)
                    otp = s_["hs"][:, 384:448]
                    nc.tensor.transpose(otp[:], s_["av_sb"][:],
                                        ident[0:64, 0:64])
                    nc.vector.tensor_copy(outT[p][:, qs], otp[:])

                def proj_half(tt, nt):
                    ts_ = slice(tt * 128, (tt + 1) * 128)
                    ns = slice(nt * 512, (nt + 1) * 512)
                    yp = ypsp.tile([128, 512], F32)
                    nc.tensor.matmul(yp[:], lhsT=outT[0][:, ts_],
                                     rhs=pwt[:, 0, ns],
                                     start=True, stop=False)
                    nc.tensor.matmul(yp[:], lhsT=outT[1][:, ts_],
                                     rhs=pwt[:, 1, ns],
                                     start=False, stop=True)
                    ys = wrk.tile([128, 512], F16, tag="ys")
                    nc.vector.tensor_copy(ys[:], yp[:])
                    nc.sync.dma_start(y_d[ts_, ns], ys[:])

                # iteration order: p inner-fast would split pairs; use
                # t = qb*2 + p so both pairs of a token tile finish on
                # consecutive t (proj after t = 4*tt+3 completes stage D)
                for w in range(NT + 3):
                    if w < NT:
                        stage_a(w)
                    if 0 <= w - 1 < NT:
                        stage_b(w - 1)
                    if 0 <= w - 2 < NT:
                        stage_c(w - 2)
                    if 0 <= w - 3 < NT:
                        td = w - 3
                        stage_d(td)
                        if td % 4 == 3:
                            proj_half(td // 4, 0)
                        elif td % 4 == 1 and td >= 5:
                            proj_half((td - 5) // 4, 1)
                proj_half(M // 2 - 1, 1)

    nc.finalize()
    return nc


def _host_prep(x, qkv_w, proj_w):
    """Per-core input maps + block selection (float64, matches fp32 ref)."""
    in_maps = []
    x64 = x.astype(np.float64)
    for core in range(NCORES):
        b = core // (NCORES // B)
        hg = core % (NCORES // B)
        heads = [hg * HPC + i for i in range(HPC)]

        xt = np.ascontiguousarray(x[b].T).astype(np.float16)

        wqkvT = np.empty((DIM, 768), np.float32)
        for p in range(2):
            hA, hB = heads[2 * p], heads[2 * p + 1]
            wqkvT[:, p*128:p*128+64] = qkv_w[hA*64:(hA+1)*64].T * SCALE
            wqkvT[:, p*128+64:p*128+128] = qkv_w[hB*64:(hB+1)*64].T * SCALE
            kbase = 256 + p * 256
            wqkvT[:, kbase:kbase+64] = qkv_w[DIM+hA*64:DIM+(hA+1)*64].T
            wqkvT[:, kbase+64:kbase+128] = qkv_w[DIM+hB*64:DIM+(hB+1)*64].T
            vbase = kbase + 128
            wqkvT[:, vbase:vbase+64] = qkv_w[2*DIM+hA*64:2*DIM+(hA+1)*64].T
            wqkvT[:, vbase+64:vbase+128] = qkv_w[2*DIM+hB*64:2*DIM+(hB+1)*64].T

        pw = np.ascontiguousarray(
            proj_w[:, heads[0]*64:(heads[-1]+1)*64].T).astype(np.float16)

        # float64 selection (matches fp32 reference ordering w/ margin)
        xb = x64[b].reshape(M, BS, DIM).mean(axis=1)
        selidx = np.zeros((1, 256), np.int32)
        wbias = np.zeros((128, 64), np.float32)
        for p in range(2):
            for hip in range(2):
                h = heads[2 * p + hip]
                qb_ = xb @ qkv_w[h*64:(h+1)*64].T.astype(np.float64)
                kb_ = xb @ qkv_w[DIM+h*64:DIM+(h+1)*64].T.astype(np.float64)
                c = qb_ @ kb_.T
                for i in range(M):
                    order = np.argsort(-c[i], kind="stable")
                    i1, i2 = int(order[0]), int(order[1])
                    col = p * 128 + i * 4 + hip * 2
                    selidx[0, col] = i1 * 64
                    selidx[0, col + 1] = i2 * 64
                    if i == i1 or i == i2:
                        wbias[hip*64:(hip+1)*64, p*32+i] = -30000.0
        in_maps.append({"xt": xt, "wq": wqkvT.astype(np.float16), "pw": pw,
                        "selidx": selidx, "wbias": wbias})
    return in_maps


def kernel(x, qkv_w, proj_w, proj_b):
    global _NC_CACHE, LAST_RESULTS
    x = np.asarray(x, np.float32)
    qkv_w = np.asarray(qkv_w, np.float32)
    proj_w = np.asarray(proj_w, np.float32)
    proj_b = np.asarray(proj_b, np.float32)

    if _NC_CACHE is None:
        _NC_CACHE = build_kernel()
    nc = _NC_CACHE

    in_maps = _host_prep(x, qkv_w, proj_w)
    res = run_bass_kernel_spmd(nc, in_maps, list(range(NCORES)))
    LAST_RESULTS = res

    out = np.zeros((B, N, DIM), np.float32)
    for core in range(NCORES):
        out[core // (NCORES // B)] += res.results[core]["y"].astype(np.float32)
    out += proj_b[None, None, :]
    return out
